# revision 11
# baseline (speedup 1.0000x reference)
"""Trainium2 Bass kernel for a 2-head MultiHeadAttn + residual + LayerNorm block.

Problem shapes (hardcoded):
  x:      [8, 2048, 384] f32      attn_mask: [8, 2048] bool (True = attend)
  qkv_w:  [384, 384] f32          qkv_b: [384] f32
  o_w:    [128, 384] f32          ln_g, ln_b: [384] f32
  out:    [8, 2048, 384] f32

Sharding: data-parallel over batch — 8 batch elements, one per NeuronCore.

Per-core dataflow (S=2048, D_model=384, H=2, Dh=64), all on-chip:
  P0  multi-queue input DMA (xT transposes spread over sync/vector/scalar
      queues, straight x on gpsimd) + PE warm-up spin so HAM reaches
      2.4 GHz before real matmuls.
  P1  K^T, Q^T projections -> qkvT [128 (h,d), 2, S] bf16 (both heads
      stacked on partitions);  V projected directly into [k, d] layout
      (lhsT = xT chunk, rhs = w_v) -> vt [128 k, c, h, 66] with a
      ones-column per head (col 64 for h0, col 65 for h1) feeding the
      softmax denominator; V chunks 4..15 interleave into the first
      attention quarter to keep PE dense while ACT ramps.
  P2  attention over 4 q-quarters (512 q each); per k-chunk c:
      - scores for BOTH heads as one row-tiled concurrent matmul pair
        (h0 rows 0-63, h1 rows 64-127) -> psum [128, 2, 512]
      - DVE cast psum -> bf16 staging (frees psum in ~300ns)
      - ACT exp over 4-chunk batches [128, 4096] (one big instruction,
        amortizing the per-instr overhead on the bottleneck engine)
      - pv matmuls (lagging exp by one batch so PE never stalls)
        accumulate [65|66, 512] psum per head; partition 64/65 =
        denominator (ones-column trick, masking exact)
  P3  per-quarter tail (overlaps next quarter's attention):
      reciprocal of denominator rows -> broadcast-DMA to 64 partitions ->
      normalization fused into the psum->attnT copy; o-proj accumulates
      both heads into ONE psum; y = po + x; LayerNorm stats via
      bn_stats/bn_aggr; rstd = exp(-0.5*ln(var+eps)) (same ACT table set
      as exp -> no table switch); scale + store on the gpsimd DMA queue.
"""

import os
import sys

import ml_dtypes
import numpy as np

for _p in ("/opt/trn_rl_repo", "/root/.axon_site/_ro/trn_rl_repo"):
    if os.path.isdir(_p) and _p not in sys.path:
        sys.path.insert(0, _p)

import concourse.bass as bass  # noqa: E402
import concourse.tile as tile  # noqa: E402
from concourse import bacc  # noqa: E402
from concourse import mybir  # noqa: E402
from concourse.bass_utils import run_bass_kernel_spmd  # noqa: E402

FP = mybir.dt.float32
BF = mybir.dt.bfloat16
AF = mybir.ActivationFunctionType
OP = mybir.AluOpType

B, S, DM = 8, 2048, 384
H, DH = 2, 64
INNER = H * DH  # 128
P = 128
SC = S // P  # 16 k-chunks of 128
DC = DM // P  # 3 model-dim chunks of 128
NQ = 4  # q-quarters
QW = S // NQ  # 512
LN_EPS = 1e-3
N_CORES = 8
SCALE = 1.0 / (DH**0.5)
EXPB = 4  # chunks per exp batch


def _build(has_mask: bool, has_bias: bool, has_affine: bool) -> bass.Bass:
    nc = bacc.Bacc(
        "TRN2", target_bir_lowering=False, debug=False, num_devices=N_CORES
    )

    xb_d = nc.dram_tensor("x_bf", [S, DM], BF, kind="ExternalInput")
    w_d = nc.dram_tensor("qkv_w_bf", [DM, 3 * INNER], BF, kind="ExternalInput")
    ow_d = nc.dram_tensor("o_w_bf", [INNER, DM], BF, kind="ExternalInput")
    mask_d = bias_d = g_d = b_d = None
    if has_mask:
        mask_d = nc.dram_tensor("mask_f", [S], FP, kind="ExternalInput")
    if has_bias:
        bias_d = nc.dram_tensor("qkv_b", [3 * INNER], FP, kind="ExternalInput")
    if has_affine:
        g_d = nc.dram_tensor("ln_g", [DM], FP, kind="ExternalInput")
        b_d = nc.dram_tensor("ln_b", [DM], FP, kind="ExternalInput")
    y_d = nc.dram_tensor("y", [S, DM], FP, kind="ExternalOutput")

    with tile.TileContext(nc) as tc:
        with tc.tile_pool(name="singles", bufs=1) as sg:
            # ---- P0: input DMA on parallel queues + PE warm-up ----
            wu = sg.tile([P, 512], BF, tag="wu")
            nc.vector.memset(wu, 0.125)

            w_sb = sg.tile([P, DC, 3 * INNER], BF, tag="w_sb")
            nc.sync.dma_start(w_sb, w_d.rearrange("(dc dp) j -> dp dc j", dp=P))
            ow_sb = sg.tile([DH, H, DM], BF, tag="ow_sb")
            nc.sync.dma_start(ow_sb, ow_d.rearrange("(h d) m -> d h m", d=DH))

            xT = sg.tile([P, DC, S], BF, tag="xT")
            qeng = [nc.sync, nc.scalar]
            qi = 0
            for st in range(4):
                for dc in range(DC):
                    qeng[qi % 2].dma_start_transpose(
                        xT[:, dc, st * 512 : (st + 1) * 512],
                        xb_d[st * 512 : (st + 1) * 512, dc * P : (dc + 1) * P],
                    )
                    qi += 1
            x_sb = sg.tile([P, SC, DM], BF, tag="x_sb")
            for c in range(SC):
                nc.gpsimd.dma_start(
                    x_sb[:, c, :],
                    xb_d.rearrange("(c p) d -> p c d", p=P)[:, c, :],
                )

            eps_sb = sg.tile([P, 1], FP, tag="eps")
            nc.vector.memset(eps_sb, LN_EPS)

            mask_sb = bias_sb = g_sb = b_sb = bco_sb = ow2_sb = None
            if mask_d is not None:
                mask_sb = sg.tile([P, SC], FP, tag="mask_sb")
                nc.sync.dma_start(mask_sb, mask_d.rearrange("(c p) -> p c", p=P))
            if bias_d is not None:
                bias_sb = sg.tile([P, 3], FP, tag="bias_sb")
                nc.sync.dma_start(bias_sb, bias_d.rearrange("(jt p) -> p jt", p=P))
                ow2_sb = sg.tile([INNER, DM], BF, tag="ow2_sb")
                nc.sync.dma_start(ow2_sb, ow_d)
            if g_d is not None and b_d is not None:
                g_sb = sg.tile([P, DM], FP, tag="g_sb")
                b_sb = sg.tile([P, DM], FP, tag="b_sb")
                nc.gpsimd.dma_start(g_sb, g_d[None, :].to_broadcast((P, DM)))
                nc.gpsimd.dma_start(b_sb, b_d[None, :].to_broadcast((P, DM)))

            qkvT = sg.tile([P, 2, S], BF, tag="qkvT")  # 0=Q^T 1=K^T
            # vt: per k-chunk, per head: [V (64) | ones col] -> pv psum
            # partition 64 = softmax denominator
            vt = sg.tile([P, SC, H, 65], BF, tag="vt")
            if mask_sb is not None:
                nc.vector.tensor_copy(vt[:, :, 0, 64:65], mask_sb[:, :, None])
                nc.vector.tensor_copy(vt[:, :, 1, 64:65], mask_sb[:, :, None])
            else:
                nc.vector.memset(vt[:, :, :, 64:65], 1.0)

            ex_raw = sg.tile([P, SC, H, QW], BF, tag="ex_raw")
            exd = sg.tile([P, SC, H, QW], BF, tag="exd")
            attnT = [
                sg.tile([DH, S], BF, tag=f"attnT{h}", name=f"attnT{h}")
                for h in range(H)
            ]
            # ones row at partition 64 = lhsT of the K=1 "broadcast matmul"
            # that spreads a denominator row across 64 psum partitions
            ones_sb = sg.tile([66, P], BF, tag="ones_sb")
            nc.vector.memset(ones_sb, 1.0)
            rr = sg.tile([66, H, S], BF, tag="rr")  # denom rows (part 64)
            rb = sg.tile([DH, H, QW], FP, tag="rb")  # 1/denom, lanes 0-63
            y_sb = sg.tile([P, SC, DM], FP, tag="y_sb")
            mv_sb = sg.tile([P, SC, 2], FP, tag="mv_sb")
            rstd_sb = sg.tile([P, SC], FP, tag="rstd_sb")
            lnv_sb = sg.tile([P, SC], FP, tag="lnv_sb")

            def v_copy(dst_c, src_ap):
                # src [128, 128] psum (both heads' V) -> vt V columns
                for h in range(H):
                    if mask_sb is not None:
                        nc.vector.tensor_scalar_mul(
                            vt[:, dst_c, h, 0:DH],
                            src_ap[:, h * DH : (h + 1) * DH],
                            mask_sb[:, dst_c : dst_c + 1],
                        )
                    else:
                        nc.vector.tensor_copy(
                            vt[:, dst_c, h, 0:DH], src_ap[:, h * DH : (h + 1) * DH]
                        )

            # ---- P1 (pre-scope): warm-up, K all, Q st0, V c0..c3 ----
            with tc.tile_pool(name="ps_pre", bufs=2, space="PSUM") as pre:
                wps = pre.tile([P, 512], FP, tag="qk", name="warm")
                for _ in range(8):
                    nc.tensor.matmul(
                        wps, lhsT=wu[:, 0:P], rhs=wu, start=True, stop=True
                    )

                def emit_qkproj(jt, st):
                    pq = pre.tile([P, 512], FP, tag="qk", name=f"qk{jt}_{st}")
                    for dc in range(DC):
                        nc.tensor.matmul(
                            pq,
                            lhsT=w_sb[:, dc, jt * P : (jt + 1) * P],
                            rhs=xT[:, dc, st * 512 : (st + 1) * 512],
                            start=(dc == 0),
                            stop=(dc == DC - 1),
                        )
                    dst = qkvT[:, jt, st * 512 : (st + 1) * 512]
                    if bias_sb is not None:
                        nc.vector.tensor_scalar_add(dst, pq, bias_sb[:, jt : jt + 1])
                    else:
                        nc.vector.tensor_copy(dst, pq)

                for st in range(4):
                    emit_qkproj(1, st)  # K first
                emit_qkproj(0, 0)  # Q for quarter 0

                for c in range(4):
                    vp = pre.tile([P, P], FP, tag="vpj", name=f"vpj{c}")
                    for dc in range(DC):
                        nc.tensor.matmul(
                            vp,
                            lhsT=xT[:, dc, c * P : (c + 1) * P],
                            rhs=w_sb[:, dc, 2 * P : 3 * P],
                            start=(dc == 0),
                            stop=(dc == DC - 1),
                        )
                    v_copy(c, vp)

                if bias_sb is not None:
                    # V-bias folded in post-normalization as + (b_v @ o_w),
                    # broadcast to 128 partitions via a K=1 ones matmul
                    bvec_bf = sg.tile([P, 1], BF, tag="bvec_bf")
                    nc.vector.tensor_copy(bvec_bf, bias_sb[:, 2:3])
                    pbv = pre.tile([P, 512], FP, tag="qk", name="pbv")
                    nc.tensor.matmul(
                        pbv[0:1, 0:DM], lhsT=bvec_bf, rhs=ow2_sb,
                        start=True, stop=True,
                    )
                    bvo_row = sg.tile([1, DM], BF, tag="bvo_row")
                    nc.vector.tensor_copy(bvo_row, pbv[0:1, 0:DM])
                    pbc = pre.tile([P, 512], FP, tag="qk", name="pbc")
                    nc.tensor.matmul(
                        pbc[:, 0:DM],
                        lhsT=ones_sb[0:1, :],
                        rhs=bvo_row,
                        start=True,
                        stop=True,
                    )
                    bco_sb = sg.tile([P, DM], FP, tag="bco_sb")
                    nc.vector.tensor_copy(bco_sb, pbc[:, 0:DM])

            # ---- P2/P3: attention + per-quarter tails ----
            with (
                tc.tile_pool(name="ps_sc", bufs=2, space="PSUM") as psc_pool,
                tc.tile_pool(name="ps_pv0", bufs=1, space="PSUM") as ppv0_pool,
                tc.tile_pool(name="ps_pv1", bufs=1, space="PSUM") as ppv1_pool,
                tc.tile_pool(name="ps_po", bufs=2, space="PSUM") as po_pool,
                tc.tile_pool(name="post", bufs=4) as post,
            ):
                y_t3 = y_d.rearrange("(c p) m -> p c m", p=P)
                ppv = [None, None]

                def emit_vproj(c):
                    vp = po_pool.tile([P, DM], FP, tag="po", name=f"vpj{c}")
                    for dc in range(DC):
                        nc.tensor.matmul(
                            vp[:, 0:P],
                            lhsT=xT[:, dc, c * P : (c + 1) * P],
                            rhs=w_sb[:, dc, 2 * P : 3 * P],
                            start=(dc == 0),
                            stop=(dc == DC - 1),
                        )
                    v_copy(c, vp[:, 0:P])

                def emit_scores(qq, c):
                    ps = psc_pool.tile([P, H, QW], FP, tag="sc", name=f"sc{qq}_{c}")
                    for h in range(H):
                        hs = slice(h * DH, (h + 1) * DH)
                        nc.tensor.matmul(
                            ps[:, h, :],
                            lhsT=qkvT[hs, 1, c * P : (c + 1) * P],
                            rhs=qkvT[hs, 0, qq * QW : (qq + 1) * QW],
                            start=True,
                            stop=True,
                        )
                    nc.vector.tensor_copy(ex_raw[:, c, :, :], ps)

                def emit_qproj_late(st):
                    ps = psc_pool.tile(
                        [P, H, QW], FP, tag="sc", name=f"qlate{st}"
                    )
                    for dc in range(DC):
                        nc.tensor.matmul(
                            ps[:, 0, :],
                            lhsT=w_sb[:, dc, 0:P],
                            rhs=xT[:, dc, st * 512 : (st + 1) * 512],
                            start=(dc == 0),
                            stop=(dc == DC - 1),
                        )
                    dst = qkvT[:, 0, st * 512 : (st + 1) * 512]
                    if bias_sb is not None:
                        nc.vector.tensor_scalar_add(dst, ps[:, 0, :], bias_sb[:, 0:1])
                    else:
                        nc.vector.tensor_copy(dst, ps[:, 0, :])

                def emit_exp(c0, c1):
                    nc.scalar.activation(
                        exd[:, c0:c1, :, :],
                        ex_raw[:, c0:c1, :, :],
                        AF.Exp,
                        scale=SCALE,
                    )

                def emit_pv(c0, c1, pv0, pv1):
                    for c in range(c0, c1):
                        for h, pv in ((0, pv0), (1, pv1)):
                            nc.tensor.matmul(
                                pv,
                                lhsT=vt[:, c, h, :],
                                rhs=exd[:, c, h, :],
                                start=(c == 0),
                                stop=(c == SC - 1),
                            )

                def emit_tail(qq, pv0, pv1):
                    q0 = qq * QW
                    for h, pv in ((0, pv0), (1, pv1)):
                        nc.vector.tensor_copy(
                            rr[64:65, h, q0 : q0 + QW], pv[64:65, :]
                        )
                    pb = psc_pool.tile([P, H, QW], FP, tag="sc", name=f"pb{qq}")
                    for h in range(H):
                        nc.tensor.matmul(
                            pb[0:DH, h, :],
                            lhsT=ones_sb[64:65, 0:DH],
                            rhs=rr[64:65, h, q0 : q0 + QW],
                            start=True,
                            stop=True,
                        )
                    nc.vector.reciprocal(rb, pb[0:DH, :, :])
                    for h, pv in ((0, pv0), (1, pv1)):
                        nc.vector.tensor_mul(
                            attnT[h][:, q0 : q0 + QW], pv[0:DH, :], rb[:, h, :]
                        )
                    for i in range(4):
                        ch = qq * 4 + i
                        po = po_pool.tile([P, DM], FP, tag="po", name=f"po{ch}")
                        for h in range(H):
                            nc.tensor.matmul(
                                po,
                                lhsT=attnT[h][:, ch * P : (ch + 1) * P],
                                rhs=ow_sb[:, h, :],
                                start=(h == 0),
                                stop=(h == H - 1),
                            )
                        yv = y_sb[:, ch, :]
                        nc.vector.tensor_add(yv, po, x_sb[:, ch, :])
                        if bco_sb is not None:
                            nc.vector.tensor_add(yv, yv, bco_sb)
                        st6 = post.tile([P, 6], FP, tag="st6")
                        nc.vector.bn_stats(st6, yv)
                        nc.vector.bn_aggr(mv_sb[:, ch, :], st6)
                    # rstd = exp(-0.5*ln(var+eps)); Ln+Exp share one ACT
                    # table set (natural_log_exp) -> no table switch
                    nc.scalar.activation(
                        lnv_sb[:, qq * 4 : qq * 4 + 4],
                        mv_sb[:, qq * 4 : qq * 4 + 4, 1],
                        AF.Ln,
                        bias=eps_sb,
                        scale=1.0,
                    )
                    nc.scalar.activation(
                        rstd_sb[:, qq * 4 : qq * 4 + 4],
                        lnv_sb[:, qq * 4 : qq * 4 + 4],
                        AF.Exp,
                        scale=-0.5,
                    )
                    for i in range(4):
                        ch = qq * 4 + i
                        o_t = post.tile([P, DM], FP, tag="o_t")
                        nc.vector.tensor_scalar(
                            o_t,
                            y_sb[:, ch, :],
                            scalar1=mv_sb[:, ch, 0:1],
                            scalar2=rstd_sb[:, ch : ch + 1],
                            op0=OP.subtract,
                            op1=OP.mult,
                        )
                        if g_sb is not None and b_sb is not None:
                            nc.vector.tensor_mul(o_t, o_t, g_sb)
                            nc.vector.tensor_add(o_t, o_t, b_sb)
                        nc.gpsimd.dma_start(y_t3[:, ch, :], o_t)

                # flat pipelined emission over all 64 (qq, c) steps; pv
                # batches lag exp by one batch (even across qq boundaries)
                # so PE never heads-of-line blocks on ACT
                pend_pv = []  # [(qq, c0, c1, pv0_ap, pv1_ap)]
                pend_tail = None  # (qq, pv0_ap, pv1_ap)
                for g in range(NQ * SC):
                    qq, c = divmod(g, SC)
                    if c == 0:
                        ppv[0] = ppv0_pool.tile(
                            [65, QW], FP, tag="pv0", name=f"pv0_{qq}"
                        )
                        ppv[1] = ppv1_pool.tile(
                            [65, QW], FP, tag="pv1", name=f"pv1_{qq}"
                        )
                    if qq == 0:
                        if 4 + c < SC:
                            emit_vproj(4 + c)
                        elif c in (12, 13, 14):
                            emit_qproj_late(c - 11)  # Q for quarters 1..3
                    emit_scores(qq, c)
                    if c % EXPB == EXPB - 1:
                        emit_exp(c - EXPB + 1, c + 1)
                        while pend_pv:
                            emit_pv(*pend_pv.pop(0))
                        if pend_tail is not None:
                            emit_tail(*pend_tail)
                            pend_tail = None
                        pend_pv.append((c - EXPB + 1, c + 1, ppv[0], ppv[1]))
                        if c == SC - 1:
                            pend_tail = (qq, ppv[0], ppv[1])
                for args in pend_pv:
                    emit_pv(*args)
                if pend_tail is not None:
                    emit_tail(*pend_tail)

    nc.compile()
    return nc


_PROGRAM_CACHE: dict = {}


def _get_program(key):
    if key not in _PROGRAM_CACHE:
        _PROGRAM_CACHE[key] = _build(*key)
    return _PROGRAM_CACHE[key]


def kernel(x, attn_mask, qkv_w, qkv_b, o_w, ln_g, ln_b, **_ignored):
    x = np.ascontiguousarray(np.asarray(x, dtype=np.float32))
    attn_mask = np.asarray(attn_mask)
    qkv_w = np.ascontiguousarray(np.asarray(qkv_w, dtype=np.float32))
    qkv_b = np.asarray(qkv_b, dtype=np.float32)
    o_w = np.ascontiguousarray(np.asarray(o_w, dtype=np.float32))
    ln_g = np.asarray(ln_g, dtype=np.float32)
    ln_b = np.asarray(ln_b, dtype=np.float32)

    has_mask = not bool(attn_mask.all())
    has_bias = bool(np.any(qkv_b != 0.0))
    has_affine = bool(np.any(ln_g != 1.0) or np.any(ln_b != 0.0))

    nc = _get_program((has_mask, has_bias, has_affine))

    mask_f = attn_mask.astype(np.float32)
    in_maps = []
    for i in range(N_CORES):
        m = {
            "x_bf": np.ascontiguousarray(x[i].astype(ml_dtypes.bfloat16)),
            "qkv_w_bf": qkv_w.astype(ml_dtypes.bfloat16),
            "o_w_bf": o_w.astype(ml_dtypes.bfloat16),
        }
        if has_mask:
            m["mask_f"] = np.ascontiguousarray(mask_f[i])
        if has_bias:
            m["qkv_b"] = qkv_b
        if has_affine:
            m["ln_g"] = ln_g
            m["ln_b"] = ln_b
        in_maps.append(m)

    trace = os.environ.get("KBENCH_TRACE", "0") == "1"
    kw = {}
    if trace:
        kw = {"trace": True, "trace_cores": [0]}
    res = run_bass_kernel_spmd(nc, in_maps, core_ids=list(range(N_CORES)), **kw)
    global LAST_RESULT
    LAST_RESULT = res
    return np.stack([res.results[i]["y"] for i in range(N_CORES)], axis=0)


LAST_RESULT = None


# revision 18
# speedup vs baseline: 1.3722x; 1.3722x over previous
"""Trainium2 Bass kernel for a 2-head MultiHeadAttn + residual + LayerNorm block.

Problem shapes (hardcoded):
  x:      [8, 2048, 384] f32      attn_mask: [8, 2048] bool (True = attend)
  qkv_w:  [384, 384] f32          qkv_b: [384] f32
  o_w:    [128, 384] f32          ln_g, ln_b: [384] f32
  out:    [8, 2048, 384] f32

Sharding: data-parallel over batch — 8 batch elements, one per NeuronCore.

Per-core dataflow (S=2048, D_model=384, H=2, Dh=64), all on-chip:
  P0  multi-queue input DMA (xT transposes spread over sync/vector/scalar
      queues, straight x on gpsimd) + PE warm-up spin so HAM reaches
      2.4 GHz before real matmuls.
  P1  K^T, Q^T projections -> qkvT [128 (h,d), 2, S] bf16 (both heads
      stacked on partitions);  V projected directly into [k, d] layout
      (lhsT = xT chunk, rhs = w_v) -> vt [128 k, c, h, 66] with a
      ones-column per head (col 64 for h0, col 65 for h1) feeding the
      softmax denominator; V chunks 4..15 interleave into the first
      attention quarter to keep PE dense while ACT ramps.
  P2  attention over 4 q-quarters (512 q each); per k-chunk c:
      - scores for BOTH heads as one row-tiled concurrent matmul pair
        (h0 rows 0-63, h1 rows 64-127) -> psum [128, 2, 512]
      - DVE cast psum -> bf16 staging (frees psum in ~300ns)
      - ACT exp over 4-chunk batches [128, 4096] (one big instruction,
        amortizing the per-instr overhead on the bottleneck engine)
      - pv matmuls (lagging exp by one batch so PE never stalls)
        accumulate [65|66, 512] psum per head; partition 64/65 =
        denominator (ones-column trick, masking exact)
  P3  per-quarter tail (overlaps next quarter's attention):
      reciprocal of denominator rows -> broadcast-DMA to 64 partitions ->
      normalization fused into the psum->attnT copy; o-proj accumulates
      both heads into ONE psum; y = po + x; LayerNorm stats via
      bn_stats/bn_aggr; rstd = exp(-0.5*ln(var+eps)) (same ACT table set
      as exp -> no table switch); scale + store on the gpsimd DMA queue.
"""

import os
import sys

import ml_dtypes
import numpy as np

for _p in ("/opt/trn_rl_repo", "/root/.axon_site/_ro/trn_rl_repo"):
    if os.path.isdir(_p) and _p not in sys.path:
        sys.path.insert(0, _p)

import concourse.bass as bass  # noqa: E402
import concourse.tile as tile  # noqa: E402
from concourse import bacc  # noqa: E402
from concourse import mybir  # noqa: E402
from concourse.bass_utils import run_bass_kernel_spmd  # noqa: E402

FP = mybir.dt.float32
BF = mybir.dt.bfloat16
AF = mybir.ActivationFunctionType
OP = mybir.AluOpType

B, S, DM = 8, 2048, 384
H, DH = 2, 64
INNER = H * DH  # 128
P = 128
SC = S // P  # 16 k-chunks of 128
DC = DM // P  # 3 model-dim chunks of 128
NQ = 4  # q-quarters
QW = S // NQ  # 512
LN_EPS = 1e-3
N_CORES = 8
SCALE = 1.0 / (DH**0.5)
EXPB = 4  # chunks per exp batch


def _build(has_mask: bool, has_bias: bool, has_affine: bool) -> bass.Bass:
    nc = bacc.Bacc(
        "TRN2", target_bir_lowering=False, debug=False, num_devices=N_CORES
    )

    xb_d = nc.dram_tensor("x_bf", [S, DM], BF, kind="ExternalInput")
    w_d = nc.dram_tensor("qkv_w_bf", [DM, 3 * INNER], BF, kind="ExternalInput")
    ow_d = nc.dram_tensor("o_w_bf", [INNER, DM], BF, kind="ExternalInput")
    mask_d = bias_d = g_d = b_d = None
    if has_mask:
        mask_d = nc.dram_tensor("mask_f", [S], FP, kind="ExternalInput")
    if has_bias:
        bias_d = nc.dram_tensor("qkv_b", [3 * INNER], FP, kind="ExternalInput")
    if has_affine:
        g_d = nc.dram_tensor("ln_g", [DM], FP, kind="ExternalInput")
        b_d = nc.dram_tensor("ln_b", [DM], FP, kind="ExternalInput")
    y_d = nc.dram_tensor("y", [S, DM], FP, kind="ExternalOutput")

    with tile.TileContext(nc) as tc:
        with tc.tile_pool(name="singles", bufs=1) as sg:
            # ---- P0: input DMA on parallel queues + PE warm-up ----
            wu = sg.tile([P, 512], BF, tag="wu")
            nc.vector.memset(wu, 0.125)

            w_sb = sg.tile([P, DC, 3 * INNER], BF, tag="w_sb")
            nc.sync.dma_start(w_sb, w_d.rearrange("(dc dp) j -> dp dc j", dp=P))
            ow_sb = sg.tile([DH, H, DM], BF, tag="ow_sb")
            nc.sync.dma_start(ow_sb, ow_d.rearrange("(h d) m -> d h m", d=DH))

            xT = sg.tile([P, DC, S], BF, tag="xT")
            qeng = [nc.sync, nc.scalar]
            qi = 0
            for st in range(4):
                for dc in range(DC):
                    qeng[qi % 2].dma_start_transpose(
                        xT[:, dc, st * 512 : (st + 1) * 512],
                        xb_d[st * 512 : (st + 1) * 512, dc * P : (dc + 1) * P],
                    )
                    qi += 1
            x_sb = sg.tile([P, SC, DM], BF, tag="x_sb")
            for c in range(SC):
                nc.gpsimd.dma_start(
                    x_sb[:, c, :],
                    xb_d.rearrange("(c p) d -> p c d", p=P)[:, c, :],
                )

            eps_sb = sg.tile([P, 1], FP, tag="eps")
            nc.vector.memset(eps_sb, LN_EPS)

            mask_sb = bias_sb = g_sb = b_sb = bco_sb = ow2_sb = None
            if mask_d is not None:
                mask_sb = sg.tile([P, SC], FP, tag="mask_sb")
                nc.sync.dma_start(mask_sb, mask_d.rearrange("(c p) -> p c", p=P))
            if bias_d is not None:
                bias_sb = sg.tile([P, 3], FP, tag="bias_sb")
                nc.sync.dma_start(bias_sb, bias_d.rearrange("(jt p) -> p jt", p=P))
                ow2_sb = sg.tile([INNER, DM], BF, tag="ow2_sb")
                nc.sync.dma_start(ow2_sb, ow_d)
            if g_d is not None and b_d is not None:
                g_sb = sg.tile([P, DM], FP, tag="g_sb")
                b_sb = sg.tile([P, DM], FP, tag="b_sb")
                nc.gpsimd.dma_start(g_sb, g_d[None, :].to_broadcast((P, DM)))
                nc.gpsimd.dma_start(b_sb, b_d[None, :].to_broadcast((P, DM)))

            qkvT = sg.tile([P, 2, S], BF, tag="qkvT")  # 0=Q^T 1=K^T
            # vt: per k-chunk, per head: [V (64) | ones col] -> pv psum
            # partition 64 = softmax denominator
            vt = sg.tile([P, SC, H, 65], BF, tag="vt")
            if mask_sb is not None:
                nc.vector.tensor_copy(vt[:, :, 0, 64:65], mask_sb[:, :, None])
                nc.vector.tensor_copy(vt[:, :, 1, 64:65], mask_sb[:, :, None])
            else:
                nc.vector.memset(vt[:, :, :, 64:65], 1.0)

            exd = sg.tile([P, SC, H, QW], BF, tag="exd")
            attnT = [
                sg.tile([DH, S], BF, tag=f"attnT{h}", name=f"attnT{h}")
                for h in range(H)
            ]
            # ones row at partition 64 = lhsT of the K=1 "broadcast matmul"
            # that spreads a denominator row across 64 psum partitions
            ones_sb = sg.tile([66, P], BF, tag="ones_sb")
            nc.vector.memset(ones_sb, 1.0)
            rr = sg.tile([66, H, S], BF, tag="rr")  # denom rows (part 64)
            rb = sg.tile([DH, H, QW], FP, tag="rb")  # 1/denom, lanes 0-63
            y_sb = sg.tile([P, SC, DM], FP, tag="y_sb")
            mv_sb = sg.tile([P, SC, 2], FP, tag="mv_sb")
            rstd_sb = sg.tile([P, SC], FP, tag="rstd_sb")
            lnv_sb = sg.tile([P, SC], FP, tag="lnv_sb")

            def v_copy(dst_c, src_ap):
                # src [128, 128] psum (both heads' V) -> vt V columns
                for h in range(H):
                    if mask_sb is not None:
                        nc.vector.tensor_scalar_mul(
                            vt[:, dst_c, h, 0:DH],
                            src_ap[:, h * DH : (h + 1) * DH],
                            mask_sb[:, dst_c : dst_c + 1],
                        )
                    else:
                        nc.vector.tensor_copy(
                            vt[:, dst_c, h, 0:DH], src_ap[:, h * DH : (h + 1) * DH]
                        )

            # ---- P1 (pre-scope): warm-up, K all, Q st0, V c0..c3 ----
            with tc.tile_pool(name="ps_pre", bufs=2, space="PSUM") as pre:
                wps = pre.tile([P, 512], FP, tag="qk", name="warm")
                for _ in range(8):
                    nc.tensor.matmul(
                        wps, lhsT=wu[:, 0:P], rhs=wu, start=True, stop=True
                    )

                def emit_qkproj(jt, st):
                    pq = pre.tile([P, 512], FP, tag="qk", name=f"qk{jt}_{st}")
                    for dc in range(DC):
                        nc.tensor.matmul(
                            pq,
                            lhsT=w_sb[:, dc, jt * P : (jt + 1) * P],
                            rhs=xT[:, dc, st * 512 : (st + 1) * 512],
                            start=(dc == 0),
                            stop=(dc == DC - 1),
                        )
                    dst = qkvT[:, jt, st * 512 : (st + 1) * 512]
                    if bias_sb is not None:
                        nc.vector.tensor_scalar_add(dst, pq, bias_sb[:, jt : jt + 1])
                    else:
                        nc.vector.tensor_copy(dst, pq)

                for st in range(4):
                    emit_qkproj(1, st)  # K first
                emit_qkproj(0, 0)  # Q for quarter 0

                for c in range(4):
                    vp = pre.tile([P, P], FP, tag="vpj", name=f"vpj{c}")
                    for dc in range(DC):
                        nc.tensor.matmul(
                            vp,
                            lhsT=xT[:, dc, c * P : (c + 1) * P],
                            rhs=w_sb[:, dc, 2 * P : 3 * P],
                            start=(dc == 0),
                            stop=(dc == DC - 1),
                        )
                    v_copy(c, vp)

                if bias_sb is not None:
                    # V-bias folded in post-normalization as + (b_v @ o_w),
                    # broadcast to 128 partitions via a K=1 ones matmul
                    bvec_bf = sg.tile([P, 1], BF, tag="bvec_bf")
                    nc.vector.tensor_copy(bvec_bf, bias_sb[:, 2:3])
                    pbv = pre.tile([P, 512], FP, tag="qk", name="pbv")
                    nc.tensor.matmul(
                        pbv[0:1, 0:DM], lhsT=bvec_bf, rhs=ow2_sb,
                        start=True, stop=True,
                    )
                    bvo_row = sg.tile([1, DM], BF, tag="bvo_row")
                    nc.vector.tensor_copy(bvo_row, pbv[0:1, 0:DM])
                    pbc = pre.tile([P, 512], FP, tag="qk", name="pbc")
                    nc.tensor.matmul(
                        pbc[:, 0:DM],
                        lhsT=ones_sb[0:1, :],
                        rhs=bvo_row,
                        start=True,
                        stop=True,
                    )
                    bco_sb = sg.tile([P, DM], FP, tag="bco_sb")
                    nc.vector.tensor_copy(bco_sb, pbc[:, 0:DM])

            # ---- P2/P3: attention + per-quarter tails ----
            with (
                tc.tile_pool(name="ps_sc", bufs=2, space="PSUM") as psc_pool,
                tc.tile_pool(name="ps_pv0", bufs=1, space="PSUM") as ppv0_pool,
                tc.tile_pool(name="ps_pv1", bufs=1, space="PSUM") as ppv1_pool,
                tc.tile_pool(name="ps_po", bufs=2, space="PSUM") as po_pool,
                tc.tile_pool(name="post", bufs=4) as post,
            ):
                y_t3 = y_d.rearrange("(c p) m -> p c m", p=P)
                ppv = [None, None]

                def emit_vproj(c):
                    vp = po_pool.tile([P, DM], FP, tag="po", name=f"vpj{c}")
                    for dc in range(DC):
                        nc.tensor.matmul(
                            vp[:, 0:P],
                            lhsT=xT[:, dc, c * P : (c + 1) * P],
                            rhs=w_sb[:, dc, 2 * P : 3 * P],
                            start=(dc == 0),
                            stop=(dc == DC - 1),
                        )
                    v_copy(c, vp[:, 0:P])

                def emit_scores(qq, c):
                    ps = psc_pool.tile([P, H, QW], FP, tag="sc", name=f"sc{qq}_{c}")
                    for h in range(H):
                        hs = slice(h * DH, (h + 1) * DH)
                        nc.tensor.matmul(
                            ps[:, h, :],
                            lhsT=qkvT[hs, 1, c * P : (c + 1) * P],
                            rhs=qkvT[hs, 0, qq * QW : (qq + 1) * QW],
                            start=True,
                            stop=True,
                        )
                    return ps

                def emit_qproj_late(st):
                    ps = psc_pool.tile(
                        [P, H, QW], FP, tag="sc", name=f"qlate{st}"
                    )
                    for dc in range(DC):
                        nc.tensor.matmul(
                            ps[:, 0, :],
                            lhsT=w_sb[:, dc, 0:P],
                            rhs=xT[:, dc, st * 512 : (st + 1) * 512],
                            start=(dc == 0),
                            stop=(dc == DC - 1),
                        )
                    dst = qkvT[:, 0, st * 512 : (st + 1) * 512]
                    if bias_sb is not None:
                        nc.vector.tensor_scalar_add(dst, ps[:, 0, :], bias_sb[:, 0:1])
                    else:
                        nc.vector.tensor_copy(dst, ps[:, 0, :])

                def emit_exp(c, ps):
                    # one big ACT instr per chunk, PSUM -> SBUF bf16
                    nc.scalar.activation(
                        exd[:, c, :, :], ps, AF.Exp, scale=SCALE
                    )

                def emit_pv(c, pv0, pv1):
                    for h, pv in ((0, pv0), (1, pv1)):
                        nc.tensor.matmul(
                            pv,
                            lhsT=vt[:, c, h, :],
                            rhs=exd[:, c, h, :],
                            start=(c == 0),
                            stop=(c == SC - 1),
                        )

                def emit_tail(qq, pv0, pv1):
                    q0 = qq * QW
                    for h, pv in ((0, pv0), (1, pv1)):
                        nc.vector.tensor_copy(
                            rr[64:65, h, q0 : q0 + QW], pv[64:65, :]
                        )
                    pb = psc_pool.tile([P, H, QW], FP, tag="sc", name=f"pb{qq}")
                    for h in range(H):
                        nc.tensor.matmul(
                            pb[0:DH, h, :],
                            lhsT=ones_sb[64:65, 0:DH],
                            rhs=rr[64:65, h, q0 : q0 + QW],
                            start=True,
                            stop=True,
                        )
                    nc.vector.reciprocal_approx_fast(rb, pb[0:DH, :, :])
                    for h, pv in ((0, pv0), (1, pv1)):
                        nc.vector.tensor_mul(
                            attnT[h][:, q0 : q0 + QW], pv[0:DH, :], rb[:, h, :]
                        )
                    for i in range(4):
                        ch = qq * 4 + i
                        po = po_pool.tile([P, DM], FP, tag="po", name=f"po{ch}")
                        for h in range(H):
                            nc.tensor.matmul(
                                po,
                                lhsT=attnT[h][:, ch * P : (ch + 1) * P],
                                rhs=ow_sb[:, h, :],
                                start=(h == 0),
                                stop=(h == H - 1),
                            )
                        yv = y_sb[:, ch, :]
                        nc.vector.tensor_add(yv, po, x_sb[:, ch, :])
                        if bco_sb is not None:
                            nc.vector.tensor_add(yv, yv, bco_sb)
                        st6 = post.tile([P, 6], FP, tag="st6")
                        nc.vector.bn_stats(st6, yv)
                        nc.vector.bn_aggr(mv_sb[:, ch, :], st6)

                def emit_final():
                    # rstd = exp(-0.5*ln(var+eps)): batched over all chunks,
                    # Ln+Exp share one ACT table set -> at most one switch
                    nc.scalar.activation(
                        lnv_sb, mv_sb[:, :, 1], AF.Ln, bias=eps_sb, scale=1.0
                    )
                    nc.scalar.activation(rstd_sb, lnv_sb, AF.Exp, scale=-0.5)
                    for ch in range(SC):
                        o_t = post.tile([P, DM], FP, tag="o_t")
                        nc.vector.tensor_scalar(
                            o_t,
                            y_sb[:, ch, :],
                            scalar1=mv_sb[:, ch, 0:1],
                            scalar2=rstd_sb[:, ch : ch + 1],
                            op0=OP.subtract,
                            op1=OP.mult,
                        )
                        if g_sb is not None and b_sb is not None:
                            nc.vector.tensor_mul(o_t, o_t, g_sb)
                            nc.vector.tensor_add(o_t, o_t, b_sb)
                        nc.gpsimd.dma_start(y_t3[:, ch, :], o_t)

                # flat pipelined emission over all 64 (qq, c) steps; pv lags
                # exp by PVLAG chunks (across qq boundaries too) so the PE
                # queue never head-of-line blocks on ACT
                PVLAG = 2
                pend_pv = []  # [(qq, c, pv0_ap, pv1_ap)]
                for g in range(NQ * SC):
                    qq, c = divmod(g, SC)
                    if c == 0:
                        ppv[0] = ppv0_pool.tile(
                            [65, QW], FP, tag="pv0", name=f"pv0_{qq}"
                        )
                        ppv[1] = ppv1_pool.tile(
                            [65, QW], FP, tag="pv1", name=f"pv1_{qq}"
                        )
                    if qq == 0:
                        if 4 + c < SC:
                            emit_vproj(4 + c)
                        elif c in (12, 13, 14):
                            emit_qproj_late(c - 11)  # Q for quarters 1..3
                    ps = emit_scores(qq, c)
                    emit_exp(c, ps)
                    pend_pv.append((qq, c, ppv[0], ppv[1]))
                    if len(pend_pv) > PVLAG:
                        pqq, pc, pv0, pv1 = pend_pv.pop(0)
                        emit_pv(pc, pv0, pv1)
                        if pc == SC - 1:
                            emit_tail(pqq, pv0, pv1)
                for pqq, pc, pv0, pv1 in pend_pv:
                    emit_pv(pc, pv0, pv1)
                    if pc == SC - 1:
                        emit_tail(pqq, pv0, pv1)
                emit_final()

    nc.compile()
    return nc


_PROGRAM_CACHE: dict = {}


def _get_program(key):
    if key not in _PROGRAM_CACHE:
        _PROGRAM_CACHE[key] = _build(*key)
    return _PROGRAM_CACHE[key]


def kernel(x, attn_mask, qkv_w, qkv_b, o_w, ln_g, ln_b, **_ignored):
    x = np.ascontiguousarray(np.asarray(x, dtype=np.float32))
    attn_mask = np.asarray(attn_mask)
    qkv_w = np.ascontiguousarray(np.asarray(qkv_w, dtype=np.float32))
    qkv_b = np.asarray(qkv_b, dtype=np.float32)
    o_w = np.ascontiguousarray(np.asarray(o_w, dtype=np.float32))
    ln_g = np.asarray(ln_g, dtype=np.float32)
    ln_b = np.asarray(ln_b, dtype=np.float32)

    has_mask = not bool(attn_mask.all())
    has_bias = bool(np.any(qkv_b != 0.0))
    has_affine = bool(np.any(ln_g != 1.0) or np.any(ln_b != 0.0))

    nc = _get_program((has_mask, has_bias, has_affine))

    mask_f = attn_mask.astype(np.float32)
    in_maps = []
    for i in range(N_CORES):
        m = {
            "x_bf": np.ascontiguousarray(x[i].astype(ml_dtypes.bfloat16)),
            "qkv_w_bf": qkv_w.astype(ml_dtypes.bfloat16),
            "o_w_bf": o_w.astype(ml_dtypes.bfloat16),
        }
        if has_mask:
            m["mask_f"] = np.ascontiguousarray(mask_f[i])
        if has_bias:
            m["qkv_b"] = qkv_b
        if has_affine:
            m["ln_g"] = ln_g
            m["ln_b"] = ln_b
        in_maps.append(m)

    trace = os.environ.get("KBENCH_TRACE", "0") == "1"
    kw = {}
    if trace:
        kw = {"trace": True, "trace_cores": [0]}
    res = run_bass_kernel_spmd(nc, in_maps, core_ids=list(range(N_CORES)), **kw)
    global LAST_RESULT
    LAST_RESULT = res
    return np.stack([res.results[i]["y"] for i in range(N_CORES)], axis=0)


LAST_RESULT = None


# revision 32
# speedup vs baseline: 1.9711x; 1.4365x over previous
"""Trainium2 Bass kernel for a 2-head MultiHeadAttn + residual + LayerNorm block.

Problem shapes (hardcoded):
  x:      [8, 2048, 384] f32      attn_mask: [8, 2048] bool (True = attend)
  qkv_w:  [384, 384] f32          qkv_b: [384] f32
  o_w:    [128, 384] f32          ln_g, ln_b: [384] f32
  out:    [8, 2048, 384] f32

Sharding: data-parallel over batch — 8 batch elements, one per NeuronCore.

Per-core dataflow (S=2048, D_model=384, H=2, Dh=64), all on-chip:
  P0  x loaded straight (4 big DMAs on the two HWDGE queues); x^T built
      on-chip with identity matmuls (regular MMs -> they also warm the
      PE HAM clock gate); weights in parallel.
  P1  K^T, Q^T projections -> qkvT [128 (h,d), 2, S] bf16 (both heads
      stacked on partitions); V projected directly into [k, d] layout
      (lhsT = xT chunk, rhs = w_v) -> vt [128 k, c, h, 65] with a
      ones-column (col 64) feeding the softmax denominator.  Only the
      work needed for the first attention steps runs up front; the rest
      (x^T blocks 4-15, K st1-3, Q st1-3, V chunks 4-15) drains as
      filler work inside quarter-0's attention loop to keep PE dense
      while ACT (the bottleneck engine) ramps.
  P2  attention over 4 q-quarters (512 q each); per k-chunk c:
      - scores for BOTH heads as one row-tiled concurrent matmul pair
        (h0 rows 0-63, h1 rows 64-127) -> psum [128, 2, 512]
      - ACT exp in ONE instr per chunk (PSUM -> SBUF bf16, 1024 cols)
      - pv matmuls (lagging exp by 2 chunks so the in-order PE queue
        never blocks on ACT) accumulate [65, 512] psum per head;
        partition 64 = denominator (ones-column trick, masking exact)
  P3  per-quarter tail, interleaved into the next quarter's steps:
      denominator rows -> K=1 ones-matmul broadcast to 64 partitions ->
      reciprocal_approx_fast -> normalization fused into the
      psum->attnT copy; o-proj accumulates both heads into ONE psum;
      y = po + x; bn_stats/bn_aggr.  Final: rstd = exp(-0.5*ln(var+eps))
      batched over all 16 chunks (Ln+Exp share one ACT table set),
      scale, stores alternating over both DMA queues.
"""

import os
import sys

import ml_dtypes
import numpy as np

for _p in ("/opt/trn_rl_repo", "/root/.axon_site/_ro/trn_rl_repo"):
    if os.path.isdir(_p) and _p not in sys.path:
        sys.path.insert(0, _p)

import concourse.bass as bass  # noqa: E402
import concourse.tile as tile  # noqa: E402
from concourse import bacc  # noqa: E402
from concourse import mybir  # noqa: E402
from concourse.bass_utils import run_bass_kernel_spmd  # noqa: E402
from concourse.masks import make_identity  # noqa: E402

FP = mybir.dt.float32
BF = mybir.dt.bfloat16
AF = mybir.ActivationFunctionType
OP = mybir.AluOpType

B, S, DM = 8, 2048, 384
H, DH = 2, 64
INNER = H * DH  # 128
P = 128
SC = S // P  # 16 k-chunks of 128
DC = DM // P  # 3 model-dim chunks of 128
NQ = 4  # q-quarters
QW = S // NQ  # 512
LN_EPS = 1e-3
N_CORES = 8
SCALE = 1.0 / (DH**0.5)


def _build(has_mask: bool, has_bias: bool, has_affine: bool) -> bass.Bass:
    nc = bacc.Bacc(
        "TRN2", target_bir_lowering=False, debug=False, num_devices=N_CORES
    )

    xb_d = nc.dram_tensor("x_bf", [S, DM], BF, kind="ExternalInput")
    w_d = nc.dram_tensor("qkv_w_bf", [DM, 3 * INNER], BF, kind="ExternalInput")
    ow_d = nc.dram_tensor("o_w_bf", [INNER, DM], BF, kind="ExternalInput")
    mask_d = bias_d = g_d = b_d = None
    if has_mask:
        mask_d = nc.dram_tensor("mask_f", [S], FP, kind="ExternalInput")
    if has_bias:
        bias_d = nc.dram_tensor("qkv_b", [3 * INNER], FP, kind="ExternalInput")
    if has_affine:
        g_d = nc.dram_tensor("ln_g", [DM], FP, kind="ExternalInput")
        b_d = nc.dram_tensor("ln_b", [DM], FP, kind="ExternalInput")
    y_d = nc.dram_tensor("y", [S, DM], FP, kind="ExternalOutput")

    with tile.TileContext(nc) as tc:
        with tc.tile_pool(name="singles", bufs=1) as sg:
            # ---- P0: inputs on both HWDGE queues + PE warm-up ----
            wu = sg.tile([P, 512], BF, tag="wu")
            nc.vector.memset(wu, 0.125)
            ident = sg.tile([P, P], BF, tag="ident")
            make_identity(nc, ident)

            # x loaded straight; first chunks on the fast HWDGE queues so
            # the on-chip transpose can start ASAP, the rest on gpsimd SWDGE
            x_sb = sg.tile([P, SC, DM], BF, tag="x_sb")
            x_r = xb_d.rearrange("(c p) d -> p c d", p=P)
            nc.sync.dma_start(x_sb[:, 0:2, :], x_r[:, 0:2, :])
            nc.scalar.dma_start(x_sb[:, 2:4, :], x_r[:, 2:4, :])
            w_sb = sg.tile([P, DC, 3 * INNER], BF, tag="w_sb")
            nc.sync.dma_start(w_sb, w_d.rearrange("(dc dp) j -> dp dc j", dp=P))
            ow_sb = sg.tile([DH, H, DM], BF, tag="ow_sb")
            nc.scalar.dma_start(ow_sb, ow_d.rearrange("(h d) m -> d h m", d=DH))
            for i in range(4, SC, 2):
                nc.gpsimd.dma_start(x_sb[:, i : i + 2, :], x_r[:, i : i + 2, :])

            xT = sg.tile([P, DC, S], BF, tag="xT")

            eps_sb = sg.tile([P, 1], FP, tag="eps")
            nc.vector.memset(eps_sb, LN_EPS)

            mask_sb = bias_sb = g_sb = b_sb = bco_sb = ow2_sb = None
            if mask_d is not None:
                mask_sb = sg.tile([P, SC], FP, tag="mask_sb")
                nc.sync.dma_start(mask_sb, mask_d.rearrange("(c p) -> p c", p=P))
            if bias_d is not None:
                bias_sb = sg.tile([P, 3], FP, tag="bias_sb")
                nc.sync.dma_start(bias_sb, bias_d.rearrange("(jt p) -> p jt", p=P))
                ow2_sb = sg.tile([INNER, DM], BF, tag="ow2_sb")
                nc.sync.dma_start(ow2_sb, ow_d)
            if g_d is not None and b_d is not None:
                g_sb = sg.tile([P, DM], FP, tag="g_sb")
                b_sb = sg.tile([P, DM], FP, tag="b_sb")
                nc.gpsimd.dma_start(g_sb, g_d[None, :].to_broadcast((P, DM)))
                nc.gpsimd.dma_start(b_sb, b_d[None, :].to_broadcast((P, DM)))

            qkvT = sg.tile([P, 2, S], BF, tag="qkvT")  # 0=Q^T 1=K^T
            # vt: per k-chunk, per head: [V (64) | ones col] -> pv psum
            # partition 64 = softmax denominator
            vt = sg.tile([P, SC, H, 65], BF, tag="vt")
            if mask_sb is not None:
                nc.vector.tensor_copy(vt[:, :, 0, 64:65], mask_sb[:, :, None])
                nc.vector.tensor_copy(vt[:, :, 1, 64:65], mask_sb[:, :, None])
            else:
                nc.vector.memset(vt[:, :, :, 64:65], 1.0)

            exd = sg.tile([P, SC, H, QW], BF, tag="exd")
            attnT = [
                sg.tile([DH, S], BF, tag=f"attnT{h}", name=f"attnT{h}")
                for h in range(H)
            ]
            # raw (unnormalized) attn copies: drain the pv psum fast so the
            # next quarter's pv accumulation never waits on the normalize
            ar_sb = sg.tile([DH, H, QW], BF, tag="ar_sb")
            # ones row at partition 64 = lhsT of the K=1 "broadcast matmul"
            # that spreads a denominator row across 64 psum partitions
            ones_sb = sg.tile([66, P], BF, tag="ones_sb")
            nc.vector.memset(ones_sb, 1.0)
            rr = sg.tile([66, H, S], BF, tag="rr")  # denom rows (part 64)
            rb = sg.tile([DH, H, QW], FP, tag="rb")  # 1/denom, lanes 0-63
            y_sb = sg.tile([P, SC, DM], FP, tag="y_sb")
            mv_sb = sg.tile([P, SC, 2], FP, tag="mv_sb")
            rstd_sb = sg.tile([P, SC], FP, tag="rstd_sb")
            # scratch for the DVE magic-rsqrt (no ACT table needed)
            I32 = mybir.dt.int32
            magic_sb = sg.tile([P, 4], I32, tag="magic_sb")
            nc.vector.memset(magic_sb, 0x5F3759DF)
            ve_sb = sg.tile([P, SC], FP, tag="ve_sb")
            ti_sb = sg.tile([P, SC], I32, tag="ti_sb")
            t1_sb = sg.tile([P, SC], FP, tag="t1_sb")
            t2_sb = sg.tile([P, SC], FP, tag="t2_sb")
            y2_sb = sg.tile([P, SC], FP, tag="y2_sb")

            def v_copy(dst_c, src_ap):
                # src [128, 128] psum (both heads' V) -> vt V columns
                for h in range(H):
                    if mask_sb is not None:
                        nc.vector.tensor_scalar_mul(
                            vt[:, dst_c, h, 0:DH],
                            src_ap[:, h * DH : (h + 1) * DH],
                            mask_sb[:, dst_c : dst_c + 1],
                        )
                    else:
                        nc.vector.tensor_copy(
                            vt[:, dst_c, h, 0:DH], src_ap[:, h * DH : (h + 1) * DH]
                        )

            def xt_block(c, tp):
                # transpose x s-block c via 3 identity matmuls into one
                # psum tile, then one strided copy into xT
                for dc in range(DC):
                    nc.tensor.matmul(
                        tp[:, dc * P : (dc + 1) * P],
                        lhsT=x_sb[:, c, dc * P : (dc + 1) * P],
                        rhs=ident,
                        start=True,
                        stop=True,
                    )
                nc.vector.tensor_copy(
                    xT[:, :, c * P : (c + 1) * P],
                    tp[:, 0:DM].rearrange("p (dc q) -> p dc q", dc=DC),
                )

            def qk_mms(jt, st, pq):
                for dc in range(DC):
                    nc.tensor.matmul(
                        pq,
                        lhsT=w_sb[:, dc, jt * P : (jt + 1) * P],
                        rhs=xT[:, dc, st * 512 : (st + 1) * 512],
                        start=(dc == 0),
                        stop=(dc == DC - 1),
                    )
                dst = qkvT[:, jt, st * 512 : (st + 1) * 512]
                if bias_sb is not None:
                    nc.vector.tensor_scalar_add(dst, pq, bias_sb[:, jt : jt + 1])
                else:
                    nc.vector.tensor_copy(dst, pq)

            def v_mms(c, vp):
                for dc in range(DC):
                    nc.tensor.matmul(
                        vp[:, 0:P],
                        lhsT=xT[:, dc, c * P : (c + 1) * P],
                        rhs=w_sb[:, dc, 2 * P : 3 * P],
                        start=(dc == 0),
                        stop=(dc == DC - 1),
                    )
                v_copy(c, vp[:, 0:P])

            # ---- P1 (pre-scope): minimum work to start attention ----
            with tc.tile_pool(name="ps_pre", bufs=3, space="PSUM") as pre:
                wps = pre.tile([P, 512], FP, tag="qk", name="warm")
                for _ in range(2):
                    nc.tensor.matmul(
                        wps, lhsT=wu[:, 0:P], rhs=wu, start=True, stop=True
                    )
                for c in range(4):
                    xt_block(c, pre.tile([P, 512], FP, tag="qk", name=f"xt{c}"))
                qk_mms(1, 0, pre.tile([P, 512], FP, tag="qk", name="k0"))
                qk_mms(0, 0, pre.tile([P, 512], FP, tag="qk", name="q0"))
                for c in range(4):
                    v_mms(c, pre.tile([P, 512], FP, tag="qk", name=f"vp{c}"))

                if bias_sb is not None:
                    # V-bias folded in post-normalization as + (b_v @ o_w),
                    # broadcast to 128 partitions via a K=1 ones matmul
                    bvec_bf = sg.tile([P, 1], BF, tag="bvec_bf")
                    nc.vector.tensor_copy(bvec_bf, bias_sb[:, 2:3])
                    pbv = pre.tile([P, 512], FP, tag="qk", name="pbv")
                    nc.tensor.matmul(
                        pbv[0:1, 0:DM], lhsT=bvec_bf, rhs=ow2_sb,
                        start=True, stop=True,
                    )
                    bvo_row = sg.tile([1, DM], BF, tag="bvo_row")
                    nc.vector.tensor_copy(bvo_row, pbv[0:1, 0:DM])
                    pbc = pre.tile([P, 512], FP, tag="qk", name="pbc")
                    nc.tensor.matmul(
                        pbc[:, 0:DM],
                        lhsT=ones_sb[0:1, :],
                        rhs=bvo_row,
                        start=True,
                        stop=True,
                    )
                    bco_sb = sg.tile([P, DM], FP, tag="bco_sb")
                    nc.vector.tensor_copy(bco_sb, pbc[:, 0:DM])

            # ---- P2/P3: attention + interleaved tails ----
            with (
                tc.tile_pool(name="ps_sc", bufs=2, space="PSUM") as psc_pool,
                tc.tile_pool(name="ps_pv0", bufs=1, space="PSUM") as ppv0_pool,
                tc.tile_pool(name="ps_pv1", bufs=1, space="PSUM") as ppv1_pool,
                tc.tile_pool(name="ps_po", bufs=2, space="PSUM") as po_pool,
                tc.tile_pool(name="post", bufs=4) as post,
            ):
                y_t3 = y_d.rearrange("(c p) m -> p c m", p=P)
                ppv = [None, None]

                def po_tile(name):
                    return po_pool.tile([P, 512], FP, tag="po", name=name)

                # deferred P1 work, drained as PE filler inside quarter 0
                # (order respects deps: T blocks before their K/V users;
                # vp(c) well before pv(c) pops at step c+2; Q st1-3 last)
                def w_xt(cc):
                    return lambda: xt_block(cc, po_tile(f"xt{cc}"))

                def w_xtk(cc, blk):
                    def f():
                        xt_block(cc, po_tile(f"xt{cc}"))
                        xt_block(cc + 1, po_tile(f"xt{cc + 1}"))
                        qk_mms(1, blk, po_tile(f"k{blk}"))

                    return f

                def w_vp(c):
                    return lambda: v_mms(c, po_tile(f"vp{c}"))

                def w_q(st):
                    return lambda: qk_mms(0, st, po_tile(f"q{st}"))

                prework = [
                    w_xt(4), w_xt(5), w_xtk(6, 1),
                    w_vp(4), w_vp(5),
                    w_xt(8), w_xt(9), w_xtk(10, 2),
                    w_vp(6), w_vp(7), w_vp(8),
                    w_xt(12), w_xt(13), w_xtk(14, 3),
                    w_vp(9), w_vp(10), w_vp(11), w_vp(12),
                    w_vp(13), w_vp(14), w_vp(15),
                    w_q(1), w_q(2), w_q(3),
                ]

                def emit_scores(qq, c):
                    ps = psc_pool.tile([P, H, QW], FP, tag="sc", name=f"sc{qq}_{c}")
                    for h in range(H):
                        hs = slice(h * DH, (h + 1) * DH)
                        nc.tensor.matmul(
                            ps[:, h, :],
                            lhsT=qkvT[hs, 1, c * P : (c + 1) * P],
                            rhs=qkvT[hs, 0, qq * QW : (qq + 1) * QW],
                            start=True,
                            stop=True,
                        )
                    return ps

                def emit_exp(c, ps):
                    nc.scalar.activation(
                        exd[:, c, :, :], ps, AF.Exp, scale=SCALE
                    )

                def emit_pv(c, pv0, pv1):
                    for h, pv in ((0, pv0), (1, pv1)):
                        nc.tensor.matmul(
                            pv,
                            lhsT=vt[:, c, h, :],
                            rhs=exd[:, c, h, :],
                            start=(c == 0),
                            stop=(c == SC - 1),
                        )

                def emit_tail_norm(qq, pv0, pv1):
                    q0 = qq * QW
                    # drain pv psum immediately (raw copies) so next
                    # quarter's pv accumulation can start right away
                    for h, pv in ((0, pv0), (1, pv1)):
                        nc.vector.tensor_copy(
                            rr[64:65, h, q0 : q0 + QW], pv[64:65, :]
                        )
                        nc.vector.tensor_copy(ar_sb[:, h, :], pv[0:DH, :])
                    pb = psc_pool.tile([P, H, QW], FP, tag="sc", name=f"pb{qq}")
                    for h in range(H):
                        nc.tensor.matmul(
                            pb[0:DH, h, :],
                            lhsT=ones_sb[64:65, 0:DH],
                            rhs=rr[64:65, h, q0 : q0 + QW],
                            start=True,
                            stop=True,
                        )
                    nc.vector.reciprocal_approx_fast(rb, pb[0:DH, :, :])
                    for h in range(H):
                        nc.vector.tensor_mul(
                            attnT[h][:, q0 : q0 + QW], ar_sb[:, h, :], rb[:, h, :]
                        )

                def emit_tail_chunk(ch):
                    po = po_tile(f"po{ch}")
                    for h in range(H):
                        nc.tensor.matmul(
                            po[:, 0:DM],
                            lhsT=attnT[h][:, ch * P : (ch + 1) * P],
                            rhs=ow_sb[:, h, :],
                            start=(h == 0),
                            stop=(h == H - 1),
                        )
                    yv = y_sb[:, ch, :]
                    nc.vector.tensor_add(yv, po[:, 0:DM], x_sb[:, ch, :])
                    if bco_sb is not None:
                        nc.vector.tensor_add(yv, yv, bco_sb)
                    st6 = post.tile([P, 6], FP, tag="st6")
                    nc.vector.bn_stats(st6, yv)
                    nc.vector.bn_aggr(mv_sb[:, ch, :], st6)

                def emit_rstd(qq):
                    # rstd = (var+eps)^-1/2 via DVE magic-seed rsqrt + two
                    # Newton iterations (keeps ACT exp-only: 1 table load)
                    s = slice(qq * 4, qq * 4 + 4)
                    ve, ti = ve_sb[:, s], ti_sb[:, s]
                    t1, t2, yy = t1_sb[:, s], t2_sb[:, s], y2_sb[:, s]
                    nc.vector.tensor_scalar_add(ve, mv_sb[:, s, 1], eps_sb)
                    nc.vector.tensor_scalar(
                        ti,
                        ve.bitcast(I32),
                        scalar1=1,
                        scalar2=None,
                        op0=OP.logical_shift_right,
                    )
                    nc.vector.tensor_sub(t1.bitcast(I32), magic_sb, ti)
                    for src, dst in ((t1, yy), (yy, rstd_sb[:, s])):
                        nc.vector.tensor_mul(t2, src, src)
                        nc.vector.tensor_mul(t2, t2, ve)
                        nc.vector.tensor_scalar(
                            t2, t2, scalar1=-0.5, scalar2=1.5,
                            op0=OP.mult, op1=OP.add,
                        )
                        nc.vector.tensor_mul(dst, src, t2)

                def emit_ts_store(ch):
                    o_t = post.tile([P, DM], FP, tag="o_t")
                    nc.vector.tensor_scalar(
                        o_t,
                        y_sb[:, ch, :],
                        scalar1=mv_sb[:, ch, 0:1],
                        scalar2=rstd_sb[:, ch : ch + 1],
                        op0=OP.subtract,
                        op1=OP.mult,
                    )
                    if g_sb is not None and b_sb is not None:
                        nc.vector.tensor_mul(o_t, o_t, g_sb)
                        nc.vector.tensor_add(o_t, o_t, b_sb)
                    eng = nc.gpsimd if ch % 2 == 0 else nc.sync
                    eng.dma_start(y_t3[:, ch, :], o_t)

                # flat pipelined emission over all 64 (qq, c) steps; pv lags
                # exp by PVLAG chunks (across qq boundaries too) so the
                # in-order PE queue never blocks on ACT; tail o-proj chunks
                # drain one-per-step behind the normalization
                PVLAG = 2
                pend_pv = []  # [(qq, c, pv0_ap, pv1_ap)]
                tailwork = []

                def queue_tail(pqq):
                    for i in range(4):
                        tailwork.append(
                            lambda ch=pqq * 4 + i: emit_tail_chunk(ch)
                        )
                    tailwork.append(lambda pqq=pqq: emit_rstd(pqq))
                    for i in range(4):
                        tailwork.append(
                            lambda ch=pqq * 4 + i: emit_ts_store(ch)
                        )

                for g in range(NQ * SC):
                    qq, c = divmod(g, SC)
                    if c == 0:
                        ppv[0] = ppv0_pool.tile(
                            [65, QW], FP, tag="pv0", name=f"pv0_{qq}"
                        )
                        ppv[1] = ppv1_pool.tile(
                            [65, QW], FP, tag="pv1", name=f"pv1_{qq}"
                        )
                    ps = emit_scores(qq, c)
                    emit_exp(c, ps)
                    pend_pv.append((qq, c, ppv[0], ppv[1]))
                    if len(pend_pv) > PVLAG:
                        pqq, pc, pv0, pv1 = pend_pv.pop(0)
                        emit_pv(pc, pv0, pv1)
                        if pc == SC - 1:
                            emit_tail_norm(pqq, pv0, pv1)
                            queue_tail(pqq)
                    if prework:
                        prework.pop(0)()
                        if prework and (c % 2 == 1):
                            prework.pop(0)()
                    elif tailwork:
                        tailwork.pop(0)()
                for pqq, pc, pv0, pv1 in pend_pv:
                    emit_pv(pc, pv0, pv1)
                    if pc == SC - 1:
                        emit_tail_norm(pqq, pv0, pv1)
                        queue_tail(pqq)
                while tailwork:
                    tailwork.pop(0)()

    nc.compile()
    return nc


_PROGRAM_CACHE: dict = {}


def _get_program(key):
    if key not in _PROGRAM_CACHE:
        _PROGRAM_CACHE[key] = _build(*key)
    return _PROGRAM_CACHE[key]


def kernel(x, attn_mask, qkv_w, qkv_b, o_w, ln_g, ln_b, **_ignored):
    x = np.ascontiguousarray(np.asarray(x, dtype=np.float32))
    attn_mask = np.asarray(attn_mask)
    qkv_w = np.ascontiguousarray(np.asarray(qkv_w, dtype=np.float32))
    qkv_b = np.asarray(qkv_b, dtype=np.float32)
    o_w = np.ascontiguousarray(np.asarray(o_w, dtype=np.float32))
    ln_g = np.asarray(ln_g, dtype=np.float32)
    ln_b = np.asarray(ln_b, dtype=np.float32)

    has_mask = not bool(attn_mask.all())
    has_bias = bool(np.any(qkv_b != 0.0))
    has_affine = bool(np.any(ln_g != 1.0) or np.any(ln_b != 0.0))

    nc = _get_program((has_mask, has_bias, has_affine))

    mask_f = attn_mask.astype(np.float32)
    in_maps = []
    for i in range(N_CORES):
        m = {
            "x_bf": np.ascontiguousarray(x[i].astype(ml_dtypes.bfloat16)),
            "qkv_w_bf": qkv_w.astype(ml_dtypes.bfloat16),
            "o_w_bf": o_w.astype(ml_dtypes.bfloat16),
        }
        if has_mask:
            m["mask_f"] = np.ascontiguousarray(mask_f[i])
        if has_bias:
            m["qkv_b"] = qkv_b
        if has_affine:
            m["ln_g"] = ln_g
            m["ln_b"] = ln_b
        in_maps.append(m)

    trace = os.environ.get("KBENCH_TRACE", "0") == "1"
    kw = {}
    if trace:
        kw = {"trace": True, "trace_cores": [0]}
    res = run_bass_kernel_spmd(nc, in_maps, core_ids=list(range(N_CORES)), **kw)
    global LAST_RESULT
    LAST_RESULT = res
    return np.stack([res.results[i]["y"] for i in range(N_CORES)], axis=0)


LAST_RESULT = None


# revision 35
# speedup vs baseline: 1.9847x; 1.0069x over previous
"""Trainium2 Bass kernel for a 2-head MultiHeadAttn + residual + LayerNorm block.

Problem shapes (hardcoded):
  x:      [8, 2048, 384] f32      attn_mask: [8, 2048] bool (True = attend)
  qkv_w:  [384, 384] f32          qkv_b: [384] f32
  o_w:    [128, 384] f32          ln_g, ln_b: [384] f32
  out:    [8, 2048, 384] f32

Sharding: data-parallel over batch — 8 batch elements, one per NeuronCore.

Per-core dataflow (S=2048, D_model=384, H=2, Dh=64), all on-chip:
  P0  x loaded straight (4 big DMAs on the two HWDGE queues); x^T built
      on-chip with identity matmuls (regular MMs -> they also warm the
      PE HAM clock gate); weights in parallel.
  P1  K^T, Q^T projections -> qkvT [128 (h,d), 2, S] bf16 (both heads
      stacked on partitions); V projected directly into [k, d] layout
      (lhsT = xT chunk, rhs = w_v) -> vt [128 k, c, h, 65] with a
      ones-column (col 64) feeding the softmax denominator.  Only the
      work needed for the first attention steps runs up front; the rest
      (x^T blocks 4-15, K st1-3, Q st1-3, V chunks 4-15) drains as
      filler work inside quarter-0's attention loop to keep PE dense
      while ACT (the bottleneck engine) ramps.
  P2  attention over 4 q-quarters (512 q each); per k-chunk c:
      - scores for BOTH heads as one row-tiled concurrent matmul pair
        (h0 rows 0-63, h1 rows 64-127) -> psum [128, 2, 512]
      - ACT exp in ONE instr per chunk (PSUM -> SBUF bf16, 1024 cols)
      - pv matmuls (lagging exp by 2 chunks so the in-order PE queue
        never blocks on ACT) accumulate [65, 512] psum per head;
        partition 64 = denominator (ones-column trick, masking exact)
  P3  per-quarter tail, interleaved into the next quarter's steps:
      denominator rows -> K=1 ones-matmul broadcast to 64 partitions ->
      reciprocal_approx_fast -> normalization fused into the
      psum->attnT copy; o-proj accumulates both heads into ONE psum;
      y = po + x; bn_stats/bn_aggr.  Final: rstd = exp(-0.5*ln(var+eps))
      batched over all 16 chunks (Ln+Exp share one ACT table set),
      scale, stores alternating over both DMA queues.
"""

import os
import sys

import ml_dtypes
import numpy as np

for _p in ("/opt/trn_rl_repo", "/root/.axon_site/_ro/trn_rl_repo"):
    if os.path.isdir(_p) and _p not in sys.path:
        sys.path.insert(0, _p)

import concourse.bass as bass  # noqa: E402
import concourse.tile as tile  # noqa: E402
from concourse import bacc  # noqa: E402
from concourse import mybir  # noqa: E402
from concourse.bass_utils import run_bass_kernel_spmd  # noqa: E402
from concourse.masks import make_identity  # noqa: E402

FP = mybir.dt.float32
BF = mybir.dt.bfloat16
AF = mybir.ActivationFunctionType
OP = mybir.AluOpType

B, S, DM = 8, 2048, 384
H, DH = 2, 64
INNER = H * DH  # 128
P = 128
SC = S // P  # 16 k-chunks of 128
DC = DM // P  # 3 model-dim chunks of 128
NQ = 4  # q-quarters
QW = S // NQ  # 512
LN_EPS = 1e-3
N_CORES = 8
SCALE = 1.0 / (DH**0.5)


def _build(has_mask: bool, has_bias: bool, has_affine: bool) -> bass.Bass:
    nc = bacc.Bacc(
        "TRN2", target_bir_lowering=False, debug=False, num_devices=N_CORES
    )

    xb_d = nc.dram_tensor("x_bf", [S, DM], BF, kind="ExternalInput")
    w_d = nc.dram_tensor("qkv_w_bf", [DM, 3 * INNER], BF, kind="ExternalInput")
    ow_d = nc.dram_tensor("o_w_bf", [INNER, DM], BF, kind="ExternalInput")
    mask_d = bias_d = g_d = b_d = None
    if has_mask:
        mask_d = nc.dram_tensor("mask_f", [S], FP, kind="ExternalInput")
    if has_bias:
        bias_d = nc.dram_tensor("qkv_b", [3 * INNER], FP, kind="ExternalInput")
    if has_affine:
        g_d = nc.dram_tensor("ln_g", [DM], FP, kind="ExternalInput")
        b_d = nc.dram_tensor("ln_b", [DM], FP, kind="ExternalInput")
    y_d = nc.dram_tensor("y", [S, DM], FP, kind="ExternalOutput")

    with tile.TileContext(nc) as tc:
        with tc.tile_pool(name="singles", bufs=1) as sg:
            # ---- P0: inputs on both HWDGE queues + PE warm-up ----
            wu = sg.tile([P, 512], BF, tag="wu")
            nc.vector.memset(wu, 0.125)
            ident = sg.tile([P, P], BF, tag="ident")
            make_identity(nc, ident)

            # x loaded straight as 16 small DMAs alternating the two fast
            # HWDGE queues (transposes can start after the first chunk);
            # weights slot in after the first four chunks
            x_sb = sg.tile([P, SC, DM], BF, tag="x_sb")
            x_r = xb_d.rearrange("(c p) d -> p c d", p=P)
            w_sb = sg.tile([P, DC, 3 * INNER], BF, tag="w_sb")
            ow_sb = sg.tile([DH, H, DM], BF, tag="ow_sb")
            for c in range(4):
                eng = nc.sync if c % 2 == 0 else nc.scalar
                eng.dma_start(x_sb[:, c, :], x_r[:, c, :])
            nc.sync.dma_start(w_sb, w_d.rearrange("(dc dp) j -> dp dc j", dp=P))
            nc.scalar.dma_start(ow_sb, ow_d.rearrange("(h d) m -> d h m", d=DH))
            for c in range(4, SC):
                eng = nc.sync if c % 2 == 0 else nc.scalar
                eng.dma_start(x_sb[:, c, :], x_r[:, c, :])

            xT = sg.tile([P, DC, S], BF, tag="xT")

            eps_sb = sg.tile([P, 1], FP, tag="eps")
            nc.vector.memset(eps_sb, LN_EPS)

            mask_sb = bias_sb = g_sb = b_sb = bco_sb = ow2_sb = None
            if mask_d is not None:
                mask_sb = sg.tile([P, SC], FP, tag="mask_sb")
                nc.sync.dma_start(mask_sb, mask_d.rearrange("(c p) -> p c", p=P))
            if bias_d is not None:
                bias_sb = sg.tile([P, 3], FP, tag="bias_sb")
                nc.sync.dma_start(bias_sb, bias_d.rearrange("(jt p) -> p jt", p=P))
                ow2_sb = sg.tile([INNER, DM], BF, tag="ow2_sb")
                nc.sync.dma_start(ow2_sb, ow_d)
            if g_d is not None and b_d is not None:
                g_sb = sg.tile([P, DM], FP, tag="g_sb")
                b_sb = sg.tile([P, DM], FP, tag="b_sb")
                nc.gpsimd.dma_start(g_sb, g_d[None, :].to_broadcast((P, DM)))
                nc.gpsimd.dma_start(b_sb, b_d[None, :].to_broadcast((P, DM)))

            qkvT = sg.tile([P, 2, S], BF, tag="qkvT")  # 0=Q^T 1=K^T
            # vt: per k-chunk, per head: [V (64) | ones col] -> pv psum
            # partition 64 = softmax denominator
            vt = sg.tile([P, SC, H, 65], BF, tag="vt")
            if mask_sb is not None:
                nc.vector.tensor_copy(vt[:, :, 0, 64:65], mask_sb[:, :, None])
                nc.vector.tensor_copy(vt[:, :, 1, 64:65], mask_sb[:, :, None])
            else:
                nc.vector.memset(vt[:, :, :, 64:65], 1.0)

            exd = sg.tile([P, SC, H, QW], BF, tag="exd")
            attnT = [
                sg.tile([DH, S], BF, tag=f"attnT{h}", name=f"attnT{h}")
                for h in range(H)
            ]
            # raw (unnormalized) attn copies: drain the pv psum fast so the
            # next quarter's pv accumulation never waits on the normalize
            ar_sb = sg.tile([DH, H, QW], BF, tag="ar_sb")
            # ones row at partition 64 = lhsT of the K=1 "broadcast matmul"
            # that spreads a denominator row across 64 psum partitions
            ones_sb = sg.tile([66, P], BF, tag="ones_sb")
            nc.vector.memset(ones_sb, 1.0)
            rr = sg.tile([66, H, S], BF, tag="rr")  # denom rows (part 64)
            rb = sg.tile([DH, H, QW], FP, tag="rb")  # 1/denom, lanes 0-63
            y_sb = sg.tile([P, SC, DM], FP, tag="y_sb")
            mv_sb = sg.tile([P, SC, 2], FP, tag="mv_sb")
            rstd_sb = sg.tile([P, SC], FP, tag="rstd_sb")
            # scratch for the DVE magic-rsqrt (no ACT table needed)
            I32 = mybir.dt.int32
            magic_sb = sg.tile([P, 4], I32, tag="magic_sb")
            nc.vector.memset(magic_sb, 0x5F3759DF)
            ve_sb = sg.tile([P, SC], FP, tag="ve_sb")
            ti_sb = sg.tile([P, SC], I32, tag="ti_sb")
            t1_sb = sg.tile([P, SC], FP, tag="t1_sb")
            t2_sb = sg.tile([P, SC], FP, tag="t2_sb")
            y2_sb = sg.tile([P, SC], FP, tag="y2_sb")
            o_sb = sg.tile([P, SC, DM], FP, tag="o_sb")

            def v_copy(dst_c, src_ap):
                # src [128, 128] psum (both heads' V) -> vt V columns
                for h in range(H):
                    if mask_sb is not None:
                        nc.vector.tensor_scalar_mul(
                            vt[:, dst_c, h, 0:DH],
                            src_ap[:, h * DH : (h + 1) * DH],
                            mask_sb[:, dst_c : dst_c + 1],
                        )
                    else:
                        nc.vector.tensor_copy(
                            vt[:, dst_c, h, 0:DH], src_ap[:, h * DH : (h + 1) * DH]
                        )

            def xt_block(c, tp):
                # transpose x s-block c via 3 identity matmuls into one
                # psum tile, then one strided copy into xT
                for dc in range(DC):
                    nc.tensor.matmul(
                        tp[:, dc * P : (dc + 1) * P],
                        lhsT=x_sb[:, c, dc * P : (dc + 1) * P],
                        rhs=ident,
                        start=True,
                        stop=True,
                    )
                nc.vector.tensor_copy(
                    xT[:, :, c * P : (c + 1) * P],
                    tp[:, 0:DM].rearrange("p (dc q) -> p dc q", dc=DC),
                )

            def qk_mms(jt, st, pq):
                for dc in range(DC):
                    nc.tensor.matmul(
                        pq,
                        lhsT=w_sb[:, dc, jt * P : (jt + 1) * P],
                        rhs=xT[:, dc, st * 512 : (st + 1) * 512],
                        start=(dc == 0),
                        stop=(dc == DC - 1),
                    )
                dst = qkvT[:, jt, st * 512 : (st + 1) * 512]
                if bias_sb is not None:
                    nc.vector.tensor_scalar_add(dst, pq, bias_sb[:, jt : jt + 1])
                else:
                    nc.vector.tensor_copy(dst, pq)

            def v_mms(c, vp):
                for dc in range(DC):
                    nc.tensor.matmul(
                        vp[:, 0:P],
                        lhsT=xT[:, dc, c * P : (c + 1) * P],
                        rhs=w_sb[:, dc, 2 * P : 3 * P],
                        start=(dc == 0),
                        stop=(dc == DC - 1),
                    )
                v_copy(c, vp[:, 0:P])

            # ---- P1 (pre-scope): minimum work to start attention ----
            with tc.tile_pool(name="ps_pre", bufs=3, space="PSUM") as pre:
                wps = pre.tile([P, 512], FP, tag="qk", name="warm")
                for _ in range(2):
                    nc.tensor.matmul(
                        wps, lhsT=wu[:, 0:P], rhs=wu, start=True, stop=True
                    )
                for c in range(4):
                    xt_block(c, pre.tile([P, 512], FP, tag="qk", name=f"xt{c}"))
                qk_mms(1, 0, pre.tile([P, 512], FP, tag="qk", name="k0"))
                qk_mms(0, 0, pre.tile([P, 512], FP, tag="qk", name="q0"))
                for c in range(4):
                    v_mms(c, pre.tile([P, 512], FP, tag="qk", name=f"vp{c}"))

                if bias_sb is not None:
                    # V-bias folded in post-normalization as + (b_v @ o_w),
                    # broadcast to 128 partitions via a K=1 ones matmul
                    bvec_bf = sg.tile([P, 1], BF, tag="bvec_bf")
                    nc.vector.tensor_copy(bvec_bf, bias_sb[:, 2:3])
                    pbv = pre.tile([P, 512], FP, tag="qk", name="pbv")
                    nc.tensor.matmul(
                        pbv[0:1, 0:DM], lhsT=bvec_bf, rhs=ow2_sb,
                        start=True, stop=True,
                    )
                    bvo_row = sg.tile([1, DM], BF, tag="bvo_row")
                    nc.vector.tensor_copy(bvo_row, pbv[0:1, 0:DM])
                    pbc = pre.tile([P, 512], FP, tag="qk", name="pbc")
                    nc.tensor.matmul(
                        pbc[:, 0:DM],
                        lhsT=ones_sb[0:1, :],
                        rhs=bvo_row,
                        start=True,
                        stop=True,
                    )
                    bco_sb = sg.tile([P, DM], FP, tag="bco_sb")
                    nc.vector.tensor_copy(bco_sb, pbc[:, 0:DM])

            # ---- P2/P3: attention + interleaved tails ----
            with (
                tc.tile_pool(name="ps_sc", bufs=2, space="PSUM") as psc_pool,
                tc.tile_pool(name="ps_pv0", bufs=1, space="PSUM") as ppv0_pool,
                tc.tile_pool(name="ps_pv1", bufs=1, space="PSUM") as ppv1_pool,
                tc.tile_pool(name="ps_po", bufs=2, space="PSUM") as po_pool,
                tc.tile_pool(name="post", bufs=4) as post,
            ):
                y_t3 = y_d.rearrange("(c p) m -> p c m", p=P)
                ppv = [None, None]

                def po_tile(name):
                    return po_pool.tile([P, 512], FP, tag="po", name=name)

                # deferred P1 work, drained as PE filler inside quarter 0
                # (order respects deps: T blocks before their K/V users;
                # vp(c) well before pv(c) pops at step c+2; Q st1-3 last)
                def w_xt(cc):
                    return lambda: xt_block(cc, po_tile(f"xt{cc}"))

                def w_xtk(cc, blk):
                    def f():
                        xt_block(cc, po_tile(f"xt{cc}"))
                        xt_block(cc + 1, po_tile(f"xt{cc + 1}"))
                        qk_mms(1, blk, po_tile(f"k{blk}"))

                    return f

                def w_vp(c):
                    return lambda: v_mms(c, po_tile(f"vp{c}"))

                def w_q(st):
                    return lambda: qk_mms(0, st, po_tile(f"q{st}"))

                prework = [
                    w_xt(4), w_xt(5), w_xtk(6, 1),
                    w_vp(4), w_vp(5),
                    w_xt(8), w_xt(9), w_xtk(10, 2),
                    w_vp(6), w_vp(7), w_vp(8),
                    w_xt(12), w_xt(13), w_xtk(14, 3),
                    w_vp(9), w_vp(10), w_vp(11), w_vp(12),
                    w_vp(13), w_vp(14), w_vp(15),
                    w_q(1), w_q(2), w_q(3),
                ]

                def emit_scores(qq, c):
                    ps = psc_pool.tile([P, H, QW], FP, tag="sc", name=f"sc{qq}_{c}")
                    for h in range(H):
                        hs = slice(h * DH, (h + 1) * DH)
                        nc.tensor.matmul(
                            ps[:, h, :],
                            lhsT=qkvT[hs, 1, c * P : (c + 1) * P],
                            rhs=qkvT[hs, 0, qq * QW : (qq + 1) * QW],
                            start=True,
                            stop=True,
                        )
                    return ps

                def emit_exp(c, ps):
                    nc.scalar.activation(
                        exd[:, c, :, :], ps, AF.Exp, scale=SCALE
                    )

                def emit_pv(c, pv0, pv1):
                    for h, pv in ((0, pv0), (1, pv1)):
                        nc.tensor.matmul(
                            pv,
                            lhsT=vt[:, c, h, :],
                            rhs=exd[:, c, h, :],
                            start=(c == 0),
                            stop=(c == SC - 1),
                        )

                def emit_tail_norm(qq, pv0, pv1):
                    q0 = qq * QW
                    # drain pv psum immediately (raw copies) so next
                    # quarter's pv accumulation can start right away
                    for h, pv in ((0, pv0), (1, pv1)):
                        nc.vector.tensor_copy(
                            rr[64:65, h, q0 : q0 + QW], pv[64:65, :]
                        )
                        nc.vector.tensor_copy(ar_sb[:, h, :], pv[0:DH, :])
                    pb = psc_pool.tile([P, H, QW], FP, tag="sc", name=f"pb{qq}")
                    for h in range(H):
                        nc.tensor.matmul(
                            pb[0:DH, h, :],
                            lhsT=ones_sb[64:65, 0:DH],
                            rhs=rr[64:65, h, q0 : q0 + QW],
                            start=True,
                            stop=True,
                        )
                    nc.vector.reciprocal_approx_fast(rb, pb[0:DH, :, :])
                    for h in range(H):
                        nc.vector.tensor_mul(
                            attnT[h][:, q0 : q0 + QW], ar_sb[:, h, :], rb[:, h, :]
                        )

                def emit_tail_chunk(ch):
                    po = po_tile(f"po{ch}")
                    for h in range(H):
                        nc.tensor.matmul(
                            po[:, 0:DM],
                            lhsT=attnT[h][:, ch * P : (ch + 1) * P],
                            rhs=ow_sb[:, h, :],
                            start=(h == 0),
                            stop=(h == H - 1),
                        )
                    yv = y_sb[:, ch, :]
                    nc.vector.tensor_add(yv, po[:, 0:DM], x_sb[:, ch, :])
                    if bco_sb is not None:
                        nc.vector.tensor_add(yv, yv, bco_sb)
                    st6 = post.tile([P, 6], FP, tag="st6")
                    nc.vector.bn_stats(st6, yv)
                    nc.vector.bn_aggr(mv_sb[:, ch, :], st6)

                def emit_rstd(qq):
                    # rstd = (var+eps)^-1/2 via DVE magic-seed rsqrt + two
                    # Newton iterations (keeps ACT exp-only: 1 table load)
                    s = slice(qq * 4, qq * 4 + 4)
                    ve, ti = ve_sb[:, s], ti_sb[:, s]
                    t1, t2, yy = t1_sb[:, s], t2_sb[:, s], y2_sb[:, s]
                    nc.vector.tensor_scalar_add(ve, mv_sb[:, s, 1], eps_sb)
                    nc.vector.tensor_scalar(
                        ti,
                        ve.bitcast(I32),
                        scalar1=1,
                        scalar2=None,
                        op0=OP.logical_shift_right,
                    )
                    nc.vector.tensor_sub(t1.bitcast(I32), magic_sb, ti)
                    for src, dst in ((t1, yy), (yy, rstd_sb[:, s])):
                        nc.vector.tensor_mul(t2, src, src)
                        nc.vector.tensor_mul(t2, t2, ve)
                        nc.vector.tensor_scalar(
                            t2, t2, scalar1=-0.5, scalar2=1.5,
                            op0=OP.mult, op1=OP.add,
                        )
                        nc.vector.tensor_mul(dst, src, t2)

                def emit_ts_store(ch):
                    o_t = o_sb[:, ch, :]
                    nc.vector.tensor_scalar(
                        o_t,
                        y_sb[:, ch, :],
                        scalar1=mv_sb[:, ch, 0:1],
                        scalar2=rstd_sb[:, ch : ch + 1],
                        op0=OP.subtract,
                        op1=OP.mult,
                    )
                    if g_sb is not None and b_sb is not None:
                        nc.vector.tensor_mul(o_t, o_t, g_sb)
                        nc.vector.tensor_add(o_t, o_t, b_sb)
                    if ch % 2 == 1:
                        eng = nc.gpsimd if ch % 4 == 1 else nc.sync
                        eng.dma_start(
                            y_t3[:, ch - 1 : ch + 1, :],
                            o_sb[:, ch - 1 : ch + 1, :],
                        )

                # flat pipelined emission over all 64 (qq, c) steps; pv lags
                # exp by PVLAG chunks (across qq boundaries too) so the
                # in-order PE queue never blocks on ACT; tail o-proj chunks
                # drain one-per-step behind the normalization
                PVLAG = 2
                pend_pv = []  # [(qq, c, pv0_ap, pv1_ap)]
                tailwork = []

                def queue_tail(pqq):
                    for i in range(4):
                        tailwork.append(
                            lambda ch=pqq * 4 + i: emit_tail_chunk(ch)
                        )
                    tailwork.append(lambda pqq=pqq: emit_rstd(pqq))
                    for i in range(4):
                        tailwork.append(
                            lambda ch=pqq * 4 + i: emit_ts_store(ch)
                        )

                for g in range(NQ * SC):
                    qq, c = divmod(g, SC)
                    if c == 0:
                        ppv[0] = ppv0_pool.tile(
                            [65, QW], FP, tag="pv0", name=f"pv0_{qq}"
                        )
                        ppv[1] = ppv1_pool.tile(
                            [65, QW], FP, tag="pv1", name=f"pv1_{qq}"
                        )
                    ps = emit_scores(qq, c)
                    emit_exp(c, ps)
                    pend_pv.append((qq, c, ppv[0], ppv[1]))
                    if len(pend_pv) > PVLAG:
                        pqq, pc, pv0, pv1 = pend_pv.pop(0)
                        emit_pv(pc, pv0, pv1)
                        if pc == SC - 1:
                            emit_tail_norm(pqq, pv0, pv1)
                            queue_tail(pqq)
                    if prework:
                        prework.pop(0)()
                        if prework and (c % 2 == 1):
                            prework.pop(0)()
                    elif tailwork:
                        tailwork.pop(0)()
                for pqq, pc, pv0, pv1 in pend_pv:
                    emit_pv(pc, pv0, pv1)
                    if pc == SC - 1:
                        emit_tail_norm(pqq, pv0, pv1)
                        queue_tail(pqq)
                while tailwork:
                    tailwork.pop(0)()

    nc.compile()
    return nc


_PROGRAM_CACHE: dict = {}


def _get_program(key):
    if key not in _PROGRAM_CACHE:
        _PROGRAM_CACHE[key] = _build(*key)
    return _PROGRAM_CACHE[key]


def kernel(x, attn_mask, qkv_w, qkv_b, o_w, ln_g, ln_b, **_ignored):
    x = np.ascontiguousarray(np.asarray(x, dtype=np.float32))
    attn_mask = np.asarray(attn_mask)
    qkv_w = np.ascontiguousarray(np.asarray(qkv_w, dtype=np.float32))
    qkv_b = np.asarray(qkv_b, dtype=np.float32)
    o_w = np.ascontiguousarray(np.asarray(o_w, dtype=np.float32))
    ln_g = np.asarray(ln_g, dtype=np.float32)
    ln_b = np.asarray(ln_b, dtype=np.float32)

    has_mask = not bool(attn_mask.all())
    has_bias = bool(np.any(qkv_b != 0.0))
    has_affine = bool(np.any(ln_g != 1.0) or np.any(ln_b != 0.0))

    nc = _get_program((has_mask, has_bias, has_affine))

    mask_f = attn_mask.astype(np.float32)
    in_maps = []
    for i in range(N_CORES):
        m = {
            "x_bf": np.ascontiguousarray(x[i].astype(ml_dtypes.bfloat16)),
            "qkv_w_bf": qkv_w.astype(ml_dtypes.bfloat16),
            "o_w_bf": o_w.astype(ml_dtypes.bfloat16),
        }
        if has_mask:
            m["mask_f"] = np.ascontiguousarray(mask_f[i])
        if has_bias:
            m["qkv_b"] = qkv_b
        if has_affine:
            m["ln_g"] = ln_g
            m["ln_b"] = ln_b
        in_maps.append(m)

    trace = os.environ.get("KBENCH_TRACE", "0") == "1"
    kw = {}
    if trace:
        kw = {"trace": True, "trace_cores": [0]}
    res = run_bass_kernel_spmd(nc, in_maps, core_ids=list(range(N_CORES)), **kw)
    global LAST_RESULT
    LAST_RESULT = res
    return np.stack([res.results[i]["y"] for i in range(N_CORES)], axis=0)


LAST_RESULT = None
